# revision 27
# baseline (speedup 1.0000x reference)
"""Trainium2 Bass kernel: causal self-attention with RoPE (B=2, T=2048, D=2048, H=16).

Sharding: 8 cores = 2-way data parallel over batch x 4-way tensor parallel over
heads.  Core c = 4*b + g computes batch b, heads 4g..4g+3, and produces a
partial output y_partial = attn_out[:, heads_g] @ w_proj[:, heads_g].T which the
host sums over g (in fp32; the device emits bf16).

All matmul operands are bf16 (psum accumulation fp32): on TRN2's PE every bf16
matmul runs at 1 cycle/row regardless of moving-dim size, which makes the
129-wide PV matmuls (ones-column softmax-denominator trick) full rate.

Per-core pipeline:
  - qkv projection per 256-token block, pass-structured (q0k0 | q1k1 | v) so
    the Activation engine drains each psum pair to SBUF right away and RoPE
    (DVE, all-bf16 for the 2x/4x modes) runs behind it.
  - scores computed transposed s^T[j,i] = k^T.T @ q^T; diagonal j-tiles trim
    their i-range to the causal half.  exp on ScalarE with the 1/sqrt(Dh)
    scale folded in; a single [128,128] lower-tri mask multiply on the
    diagonal 128-chunk only.
  - PV: lhsT = p~^T chunk, rhs = [v | ones] so psum column 128 is the softmax
    denominator; per-partition reciprocal multiply normalizes; a PE transpose
    turns o into o^T feeding the output projection as lhsT.
  - PSUM budget (8 banks): tags qk/v/s/pv, 2 bufs each.  q|k pairs, v t-chunk
    pairs and pv ic-pairs are packed two accumulators per bank via the
    has_written overwrite trick (start=True only on the bank's first group).
  - Emission interleaves attention of pair p with the qkv of pair p+1 and the
    output projection, so the Act-bound exp phases hide under PE-bound matmul
    phases.  DMAs are batched (one per 256-token x block, per-head weights,
    per-128-row y tile) to bound the shared HWDGE descriptor-gen cost.
"""

import sys

import numpy as np
import ml_dtypes

for _p in ("/opt/trn_rl_repo", "/root/.axon_site/_ro/trn_rl_repo"):
    if _p not in sys.path:
        sys.path.append(_p)

import concourse.bass as bass  # noqa: F401
import concourse.bacc as bacc
import concourse.tile as tile
from concourse import mybir
from concourse.bass_utils import run_bass_kernel_spmd

F32 = mybir.dt.float32
BF16 = mybir.dt.bfloat16
AF = mybir.ActivationFunctionType
BF_NP = ml_dtypes.bfloat16

B, T, D, H = 2, 2048, 2048, 16
HPC = H // 4   # heads per core (4-way head TP)
DH = D // H    # 128
SCALE = float(DH) ** -0.5
TBE = 256      # qkv projection t-block
SB = 512       # attention i-block
ND = D // 128  # contraction d-tiles


def _schedule(t):
    """Emission order as a list of unit tuples."""
    NTB = t // TBE
    NSB = t // SB
    NQ = (t // 128) // 4  # proj quarters (4 t-tiles each)
    if t == 2048:
        u = [("w", 0), ("qkv", 0, 0), ("qkv", 0, 1),
             ("qkv", 0, 2), ("w", 1), ("qkv", 0, 3)]
        # pair-0 attention rounds with pair-0 tail + pair-1 qkv as PE filler
        fill = [("qkv", 0, 4), ("qkv", 0, 5), ("qkv", 0, 6), ("qkv", 0, 7),
                ("qkv", 1, 0), ("qkv", 1, 1), ("qkv", 1, 2), ("qkv", 1, 3)]
        for k in range(4):
            u += [("sc", 0, k), fill[2 * k], ("pv", 0, k),
                  ("sc", 1, k), fill[2 * k + 1], ("pv", 1, k)]
        fill = [("qkv", 1, 4), ("qkv", 1, 5), ("qkv", 1, 6), ("qkv", 1, 7),
                ("projh", 0, 0), ("projh", 0, 1), ("projh", 1, 0), ("projh", 1, 1)]
        for k in range(4):
            u += [("sc", 2, k), fill[2 * k], ("pv", 2, k),
                  ("sc", 3, k), fill[2 * k + 1], ("pv", 3, k)]
        u += [("projh", 2, 0), ("projh", 2, 1), ("projh", 3, 0), ("projh", 3, 1)]
        return u
    # generic sequential fallback (used for small-t simulation checks)
    u = [("w", 0), ("qkv", 0, 0)]
    for tb in range(1, NTB):
        u.append(("qkv", 0, tb))
    for par in range(2):
        for ib in range(NSB):
            u += [("sc", par, ib), ("pv", par, ib)]
    u.append(("w", 1))
    for tb in range(NTB):
        u.append(("qkv", 1, tb))
    for par in range(2):
        h = 2 + par
        for ib in range(NSB):
            u += [("sc", h, ib), ("pv", h, ib)]
    for q in range(NQ):
        u += [("projh", q, 0), ("projh", q, 1)]
    return u


def build_nc(t=T, **_ignored):
    NT = t // 128    # token tiles
    NSB = t // SB

    nc = bacc.Bacc("TRN2", target_bir_lowering=False, debug=False)

    xb = nc.dram_tensor("xb", [128, ND * t], BF16, kind="ExternalInput").ap()
    wqh = nc.dram_tensor("wqh", [128, HPC * D], BF16, kind="ExternalInput").ap()
    wkh = nc.dram_tensor("wkh", [128, HPC * D], BF16, kind="ExternalInput").ap()
    wvh = nc.dram_tensor("wvh", [128, 2 * 2 * D], BF16, kind="ExternalInput").ap()
    wph = nc.dram_tensor("wph", [128, HPC * D], BF16, kind="ExternalInput").ap()
    cosb = nc.dram_tensor("cosb", [DH, t], BF16, kind="ExternalInput").ap()
    sinmb = nc.dram_tensor("sinmb", [DH, t], BF16, kind="ExternalInput").ap()
    trim = nc.dram_tensor("trim", [128, 128], BF16, kind="ExternalInput").ap()
    idm = nc.dram_tensor("idm", [128, 128], BF16, kind="ExternalInput").ap()
    y = nc.dram_tensor("y", [t, D], BF16, kind="ExternalOutput").ap()

    xb3 = xb.rearrange("p (a tt) -> p a tt", a=ND)

    units = _schedule(t)
    qkv_seq = [u for u in units if u[0] == "qkv"]

    with tile.TileContext(nc) as tc:
        with (
            tc.tile_pool(name="consts", bufs=1) as cpool,
            tc.tile_pool(name="oTp", bufs=1) as opool,
            tc.tile_pool(name="qksb", bufs=2) as qkpool,
            tc.tile_pool(name="wp", bufs=2) as wpool,
            tc.tile_pool(name="vep", bufs=2) as vep,
            tc.tile_pool(name="xtp", bufs=2) as xtp,
            tc.tile_pool(name="qrawp", bufs=6) as qrawp,
            tc.tile_pool(name="ptp", bufs=20) as ptp,
            tc.tile_pool(name="tmpp", bufs=4) as tmpp,
            tc.tile_pool(name="smallp", bufs=4) as smallp,
            tc.tile_pool(name="ystp", bufs=2) as ystp,
            tc.tile_pool(name="ps", bufs=2, space="PSUM") as psp,
        ):
            st = {}  # emitter state

            def emit_consts():
                # Emitted after the first qkv unit: the Act queue head then
                # holds that unit's psum-drain copies, so these triggers fire
                # (and hit the shared DMA-engine FIFO) only ~8us in, keeping
                # the startup FIFO clear for the first weights/x tiles.
                cos_sb = cpool.tile([DH, t], BF16, tag="cos", name="cos_sb")
                nc.sync.dma_start(cos_sb[:], cosb[:])
                sin_sb = cpool.tile([DH, t], BF16, tag="sin", name="sin_sb")
                nc.sync.dma_start(sin_sb[:], sinmb[:])
                tri_sb = cpool.tile([128, 128], BF16, tag="tri", name="tri_sb")
                nc.sync.dma_start(tri_sb[:], trim[:])
                id_sb = cpool.tile([128, 128], BF16, tag="id", name="id_sb")
                nc.sync.dma_start(id_sb[:], idm[:])
                st["cos"], st["sin"] = cos_sb, sin_sb
                st["tri"], st["id"] = tri_sb, id_sb
                st["oT"] = [
                    opool.tile([DH, t], BF16, tag=f"oT{h}", name=f"oT{h}")
                    for h in range(HPC)
                ]

            def emit_weights(pair):
                # All input DMAs ride the SP queue in need order; a DMA
                # trigger holds its sequencer ~650ns (DGE delay), and putting
                # them on the Act queue would delay the psum-drain copies.
                eng = nc.sync
                for par in range(2):
                    h = 2 * pair + par
                    first = pair == 0 and par == 0
                    # the very first q/k weights stream in halves, interleaved
                    # with the first x chunks, so the opening matmuls start
                    # ~3us earlier on the serialized DMA engine
                    nck = 2 if first else 1
                    wq = wpool.tile([128, D], BF16, tag=f"wq{par}", name=f"wq{h}")
                    wk = wpool.tile([128, D], BF16, tag=f"wk{par}", name=f"wk{h}")
                    for ci in range(nck):
                        c0, c1 = D * ci // nck, D * (ci + 1) // nck
                        eng.dma_start(wq[:, c0:c1], wqh[:, D * h + c0:D * h + c1])
                        eng.dma_start(wk[:, c0:c1], wkh[:, D * h + c0:D * h + c1])
                        if first:
                            emit_xt_dma(*qkv_seq[0][1:], nchunks=2,
                                        only_chunk=ci)
                    st[("wq", pair, par)] = wq
                    st[("wk", pair, par)] = wk
                wv = wpool.tile([128, 2 * D], BF16, tag="wv", name=f"wv{pair}")
                eng.dma_start(wv[:], wvh[:, 2 * D * pair:2 * D * (pair + 1)])
                st[("wv", pair)] = wv
                if pair == 1:
                    wp_sb = cpool.tile([128, HPC * D], BF16, tag="wpj", name="wp_sb")
                    nc.sync.dma_start(wp_sb[:], wph[:])
                    st["wp"] = wp_sb
                for par in range(2):
                    q_sb = qkpool.tile([DH, t], BF16, tag=f"q{par}", name=f"q{pair}_{par}")
                    k_sb = qkpool.tile([DH, t], BF16, tag=f"k{par}", name=f"k{pair}_{par}")
                    st[("q", pair, par)] = q_sb
                    st[("k", pair, par)] = k_sb
                    ve = vep.tile([128, NT * 129], BF16, tag=f"ve{par}", name=f"ve{pair}_{par}")
                    nc.vector.memset(ve[:], 1.0)
                    st[("ve", pair, par)] = ve

            def emit_xt_dma(pair, tb, nchunks=1, only_chunk=None):
                key = ("xt", pair, tb)
                if only_chunk is None or only_chunk == 0:
                    st[key] = xtp.tile([128, ND * TBE], BF16, tag="xt",
                                       name=f"xt{pair}_{tb}")
                xt = st[key]
                t0 = TBE * tb
                xt3 = xt.rearrange("p (a tt) -> p a tt", a=ND)
                step = ND // nchunks
                for ci, a0 in enumerate(range(0, ND, step)):
                    if only_chunk is not None and ci != only_chunk:
                        continue
                    nc.sync.dma_start(
                        xt3[:, a0:a0 + step, :],
                        xb3[:, a0:a0 + step, t0:t0 + TBE],
                    )

            def rope(src, dst, tb, name):
                """dst[:, tb block] = src*cos + rotate_half(src)*sin, all bf16.
                rows 0:64 of sin hold +sin, rows 64:128 hold -sin."""
                t0, t1 = TBE * tb, TBE * (tb + 1)
                cos_sb, sin_sb = st["cos"], st["sin"]
                r1 = tmpp.tile([128, TBE], BF16, tag="r1", name=f"r1_{name}")
                nc.vector.tensor_mul(r1[:], src[:], cos_sb[:, t0:t1])
                # both DVE inputs must share a base partition (SBUF+SBUF
                # rule), so sin rows 64:128 hold -sin and rows 0:64 hold +sin
                r2 = tmpp.tile([128, TBE], BF16, tag="r2", name=f"r2_{name}")
                nc.vector.tensor_mul(r2[0:64, :], src[64:128, :], sin_sb[64:128, t0:t1])
                nc.vector.tensor_mul(r2[64:128, :], src[0:64, :], sin_sb[0:64, t0:t1])
                nc.vector.tensor_add(dst[:, t0:t1], r1[:], r2[:])

            def emit_qkv(pair, tb, seq_i, mid_hook=None):
                if seq_i + 1 < len(qkv_seq):
                    _, npair, ntb = qkv_seq[seq_i + 1]
                    emit_xt_dma(npair, ntb)
                xt = st.pop(("xt", pair, tb))
                t0 = TBE * tb
                for par in range(2):
                    wq, wk = st[("wq", pair, par)], st[("wk", pair, par)]
                    qk = psp.tile([128, 2 * TBE], F32, tag="qk", name=f"qk{pair}_{tb}_{par}")
                    for di in range(ND):
                        d0, d1 = 128 * di, 128 * (di + 1)
                        xsl = xt[:, TBE * di:TBE * (di + 1)]
                        nc.tensor.matmul(qk[:, 0:TBE], wq[:, d0:d1], xsl,
                                         start=(di == 0), stop=(di == ND - 1),
                                         skip_group_check=True)
                        nc.tensor.matmul(qk[:, TBE:2 * TBE], wk[:, d0:d1], xsl,
                                         start=False, stop=(di == ND - 1),
                                         skip_group_check=True)
                    qraw = qrawp.tile([128, TBE], BF16, tag="qraw", name=f"qr{pair}_{tb}_{par}")
                    nc.scalar.copy(qraw[:], qk[:, 0:TBE])
                    kraw = qrawp.tile([128, TBE], BF16, tag="qraw", name=f"kr{pair}_{tb}_{par}")
                    nc.scalar.copy(kraw[:], qk[:, TBE:2 * TBE])
                    if mid_hook is not None:
                        mid_hook()
                        mid_hook = None
                    rope(qraw, st[("q", pair, par)], tb, f"q{pair}_{tb}_{par}")
                    rope(kraw, st[("k", pair, par)], tb, f"k{pair}_{tb}_{par}")
                # v pass: out [t-chunk, dh_even | dh_odd] per 128-t chunk
                wv = st[("wv", pair)]
                vps = psp.tile([128, 2 * TBE], F32, tag="v", name=f"v{pair}_{tb}")
                for di in range(ND):
                    for tt in range(TBE // 128):
                        nc.tensor.matmul(
                            vps[:, 256 * tt:256 * (tt + 1)],
                            xt[:, TBE * di + 128 * tt:TBE * di + 128 * (tt + 1)],
                            wv[:, 256 * di:256 * (di + 1)],
                            start=(di == 0 and tt == 0), stop=(di == ND - 1),
                            skip_group_check=True)
                # GPSIMD cannot read PSUM on hw: v drains ride the DVE
                for tt in range(TBE // 128):
                    gt = (TBE // 128) * tb + tt
                    for par in range(2):
                        nc.vector.tensor_copy(
                            st[("ve", pair, par)][:, 129 * gt:129 * gt + 128],
                            vps[:, 256 * tt + 128 * par:256 * tt + 128 * (par + 1)])

            def emit_scores(h, ib):
                pair, par = divmod(h, 2)
                q_sb, k_sb = st[("q", pair, par)], st[("k", pair, par)]
                i0 = SB * ib
                jt_max = (i0 + SB) // 128 - 1
                pts = {}
                for jt in range(jt_max + 1):
                    m = jt - (SB // 128) * ib
                    c0 = max(0, 128 * m)  # causal column trim within i-block
                    w = SB - c0
                    s_ps = psp.tile([128, SB], F32, tag="s", name=f"s{h}_{ib}_{jt}")
                    nc.tensor.matmul(
                        s_ps[:, 0:w],
                        k_sb[:, 128 * jt:128 * (jt + 1)],
                        q_sb[:, i0 + c0:i0 + SB],
                        start=True, stop=True)
                    pt = ptp.tile([128, SB], BF16, tag="pt", name=f"pt{h}_{ib}_{jt}")
                    nc.scalar.activation(pt[:, c0:SB], s_ps[:, 0:w], AF.Exp, scale=SCALE)
                    if m >= 0:  # diagonal tile: lower-tri mask on the 128-chunk
                        nc.vector.tensor_mul(pt[:, c0:c0 + 128], pt[:, c0:c0 + 128],
                                             st["tri"][:])
                    pts[jt] = pt
                st[("pts", h)] = pts

            def emit_pv(h, ib):
                pair, par = divmod(h, 2)
                ve = st[("ve", pair, par)]
                pts = st.pop(("pts", h))
                i0 = SB * ib
                nic = SB // 128
                pvt = [psp.tile([128, 258], F32, tag="pv", name=f"pv{h}_{ib}_{half}")
                       for half in range(nic // 2)]
                for ic in range(nic):
                    acc = pvt[ic // 2][:, 129 * (ic % 2):129 * (ic % 2) + 129]
                    last = nic * ib + ic
                    for jt in range(last + 1):
                        nc.tensor.matmul(
                            acc,
                            pts[jt][:, 128 * ic:128 * (ic + 1)],
                            ve[:, 129 * jt:129 * (jt + 1)],
                            start=(ic % 2 == 0 and jt == 0), stop=(jt == last),
                            skip_group_check=True)
                o_sbs = []
                for ic in range(nic):
                    sl = pvt[ic // 2][:, 129 * (ic % 2):129 * (ic % 2) + 129]
                    rc = smallp.tile([128, 1], F32, tag="rc", name=f"rc{h}_{ib}_{ic}")
                    nc.vector.reciprocal(rc[:], sl[:, 128:129])
                    o_sb = smallp.tile([128, 128], BF16, tag="o", name=f"o{h}_{ib}_{ic}")
                    nc.vector.tensor_scalar_mul(o_sb[:], sl[:, 0:128], rc[:])
                    o_sbs.append(o_sb)
                for ic in range(nic):
                    ot = psp.tile([128, 128], BF16, tag="pv", name=f"ot{h}_{ib}_{ic}")
                    nc.tensor.transpose(ot[:], o_sbs[ic][:], st["id"][:])
                    c0 = i0 + 128 * ic
                    nc.scalar.copy(st["oT"][h][:, c0:c0 + 128], ot[:])

            def emit_projh(q, half, last=False):
                wp_sb, oT = st["wp"], st["oT"]
                for tt in range(4 * q + 2 * half, 4 * q + 2 * half + 2):
                    yst = ystp.tile([128, D], BF16, tag="yst", name=f"yst{tt}")
                    for db in range(D // 512):
                        yp = psp.tile([128, 512], F32, tag="qk", name=f"yp{tt}_{db}")
                        for hh in range(HPC):
                            nc.tensor.matmul(
                                yp[:],
                                oT[hh][:, 128 * tt:128 * (tt + 1)],
                                wp_sb[:, D * hh + 512 * db:D * hh + 512 * (db + 1)],
                                start=(hh == 0), stop=(hh == HPC - 1))
                        nc.vector.tensor_copy(yst[:, 512 * db:512 * (db + 1)], yp[:])
                        if last:
                            # stream the tail out per 512-col chunk so the
                            # final DMA isn't serialized behind all 4 copies
                            nc.sync.dma_start(
                                y[128 * tt:128 * (tt + 1), 512 * db:512 * (db + 1)],
                                yst[:, 512 * db:512 * (db + 1)])
                    if not last:
                        nc.sync.dma_start(y[128 * tt:128 * (tt + 1), :], yst[:])

            # ---- emit the program ----
            qkv_i = 0
            for unit in units:
                kind = unit[0]
                if kind == "w":
                    emit_weights(unit[1])
                elif kind == "qkv":
                    # first qkv unit emits the consts between its psum drain
                    # and its rope, keeping the startup DMA FIFO clear
                    emit_qkv(unit[1], unit[2], qkv_i,
                             mid_hook=emit_consts if qkv_i == 0 else None)
                    qkv_i += 1
                elif kind == "sc":
                    emit_scores(unit[1], unit[2])
                elif kind == "pv":
                    emit_pv(unit[1], unit[2])
                elif kind == "projh":
                    emit_projh(unit[1], unit[2], last=(unit is units[-1]))

    nc.compile()
    return nc


def host_consts(t=T):
    """RoPE cos / sign-folded sin (bf16), lower-tri mask, identity."""
    inv = (1.0 / (np.float32(10000.0) ** (np.arange(0, DH, 2, dtype=np.float32) / np.float32(DH)))).astype(np.float32)
    tt = np.arange(t, dtype=np.float32)
    fr = np.outer(tt, inv).astype(np.float32)       # [t, 64]
    emb = np.concatenate([fr, fr], axis=1)          # [t, 128]
    cosb = np.ascontiguousarray(np.cos(emb).T).astype(BF_NP)
    sinT = np.sin(emb).T.astype(np.float32)
    sinmb = np.ascontiguousarray(np.concatenate([sinT[64:], -sinT[:64]], axis=0)).astype(BF_NP)
    jj = np.arange(128)[:, None]
    cc = np.arange(128)[None, :]
    trim = (cc >= jj).astype(BF_NP)
    idm = np.eye(128, dtype=np.float32).astype(BF_NP)
    return cosb, sinmb, trim, idm


def _warrange(w):
    """[128*nh rows, D] head-major weight slice -> [128, nh*D] sbuf layout:
    block h, col di*128+c of partition p  =  w[128*h + c, 128*di + p]."""
    nh = w.shape[0] // 128
    d = w.shape[1]
    out = np.empty((128, nh * d), dtype=np.float32)
    for h in range(nh):
        a = w[128 * h:128 * (h + 1), :].T.reshape(d // 128, 128, 128)  # [di, p, c]
        out[:, d * h:d * (h + 1)] = a.transpose(1, 0, 2).reshape(128, d)
    return out.astype(BF_NP)


def _wvarrange(w):
    """[512 rows, D] 4-head v-weights -> [128, 2*2*D]: per pair, di-major blocks
    of [even-head 128 cols | odd-head 128 cols]."""
    d = w.shape[1]
    blocks = []
    for p2 in range(2):
        e = w[256 * p2:256 * p2 + 128, :].T.reshape(d // 128, 128, 128)
        o = w[256 * p2 + 128:256 * p2 + 256, :].T.reshape(d // 128, 128, 128)
        pair = np.concatenate([e, o], axis=2)          # [di, p, 256]
        blocks.append(pair.transpose(1, 0, 2).reshape(128, 2 * d))
    return np.concatenate(blocks, axis=1).astype(BF_NP)


def shard_inputs(x, w_qkv, w_proj, t=T):
    """Build the 8 per-core input maps (bf16)."""
    cosb, sinmb, trim, idm = host_consts(t)
    d = x.shape[2]
    xbs = []
    for b in range(B):
        xT = np.ascontiguousarray(x[b].T)                    # [D, t]
        xbs.append(np.ascontiguousarray(
            xT.reshape(ND, 128, t).transpose(1, 0, 2).reshape(128, ND * t)
        ).astype(BF_NP))
    in_maps = []
    for c in range(8):
        b, g = divmod(c, 4)
        s0, s1 = 512 * g, 512 * (g + 1)
        wp = w_proj[:, s0:s1].T                               # [512, D]
        wph = np.ascontiguousarray(
            wp.reshape(HPC, 128, d).transpose(1, 0, 2).reshape(128, HPC * d)
        ).astype(BF_NP)
        in_maps.append(dict(
            xb=xbs[b],
            wqh=_warrange(w_qkv[s0:s1, :]),
            wkh=_warrange(w_qkv[d + s0:d + s1, :]),
            wvh=_wvarrange(w_qkv[2 * d + s0:2 * d + s1, :]),
            wph=wph,
            cosb=cosb, sinmb=sinmb, trim=trim, idm=idm,
        ))
    return in_maps


_NC_CACHE = {}


def get_nc(t=T, **kw):
    if t not in _NC_CACHE:
        _NC_CACHE[t] = build_nc(t=t)
    return _NC_CACHE[t]


def kernel(x, w_qkv, w_proj):
    x = np.asarray(x, dtype=np.float32)
    w_qkv = np.asarray(w_qkv, dtype=np.float32)
    w_proj = np.asarray(w_proj, dtype=np.float32)
    b_, t_, d_ = x.shape
    in_maps = shard_inputs(x, w_qkv, w_proj, t=t_)
    nc = get_nc(t=t_)
    res = run_bass_kernel_spmd(nc, in_maps, list(range(8))).results
    out = np.zeros((b_, t_, d_), dtype=np.float32)
    for c in range(8):
        b, _ = divmod(c, 4)
        out[b] += res[c]["y"].astype(np.float32)
    return out
